# revision 33
# baseline (speedup 1.0000x reference)
"""Self-attention (8 heads, d=64, B=2, N=4096, D=512) on 8 TRN2 NeuronCores.

Sharding: batch*heads across cores — core c handles batch b=c//4, heads
(2*(c%4), 2*(c%4)+1). Projection weights are sliced per-core on the host;
x is pre-transposed on the host so the device needs no transposes at all.

v2 changes vs baseline (403us -> target ~260us):
  - softmax exp split across two engines: ACT does exact exp (bf16 out),
    DVE does a Schraudolph-style exp for kc%5==4 tiles (one fused
    multiply-add writing int16 bf16-bit-patterns; the uniform half-ulp
    bias cancels in the softmax ratio). Measured full-model rel err
    ~7e-3 vs the 2e-2 gate.
  - h-outer attention loop: one live av accumulator (2 PSUM banks)
    frees banks for a dedicated out-projection pool.
  - out-projection runs per-qq inside the loop, DMAs PSUM->DRAM
    directly (no SBUF staging copy).
  - projection-phase PSUM->SBUF copies spread across ACT and DVE.
  - input DMA chunked so projections start after the first chunk.

Device dataflow (per core, fully transposed "scoresT" formulation):
  qT2/kT2 [hd=128, n]  = W.T-chunks @ xT-chunks          (PE)
  v2      [n, hd+ones] natural                            (PE, bf16)
  per head h, per q-chunk qq (1024 wide):
    for kc in 32:  scT psum[128k,1024q] = kh.T @ qh       (PE)
                   attnT = exp(scT*SCALE) -> bf16 SBUF    (ACT or DVE)
                   av[65,1024] += v2'[kc].T @ attnT       (PE, accumulate)
    row 64 of av = softmax denominator (ones column of v2')
    outT[h] = av[:64] * (1/denom)                         (DVE + DMA bcast)
  out[qq] = sum_h outT[h].T @ woT[h]  -> DMA PSUM->DRAM   (PE)
Host: out[b] = sum of its 4 cores' partials + bo.
"""
import math
import numpy as np
import ml_dtypes
from contextlib import ExitStack

import concourse.bass as bass
from concourse import bacc
import concourse.mybir as mybir
import concourse.tile as tile
from concourse.bass_utils import run_bass_kernel_spmd

B, N, D = 2, 4096, 512
HEADS, DH = 8, 64
SCALE = DH ** -0.5

F32 = mybir.dt.float32
F32R = mybir.dt.float32r
BF16 = mybir.dt.bfloat16
I16 = mybir.dt.int16

QQ_W = 1024          # q-chunk width in the attention loop
N_QQ = N // QQ_W     # 4
N_KC = N // 128      # 32 key chunks
DCH = D // 128       # 4 contraction chunks for projections

# Schraudolph exp in bf16 bit space: bits = y*128*log2(e) + (128*127 - s + 0.5)
# with y = score*SCALE folded into the multiplier. s = 128*0.0579 balances
# the max relative error of the piecewise-linear approximation (~±3%).
EXP_A = 128.0 * math.log2(math.e) * SCALE
EXP_B = 128.0 * 127.0 - 128.0 * 0.0579 + 0.5


def dve_exp_tile(kc: int, s: int, n_s: int) -> bool:
    """Which score half-tiles get the approximate DVE exp. Normally the s=1
    half of each kc (minus every 16th, rebalancing DVE's normalize work);
    single-half windows alternate by kc instead."""
    if n_s == 1:
        return kc % 2 == 1
    return s == 1 and kc % 16 != 15


def build_bass():
    nc = bacc.Bacc(None, target_bir_lowering=False)

    xT = nc.dram_tensor("xT", [DCH, 128, N], BF16, kind="ExternalInput")
    wqT = nc.dram_tensor("wqT", [D, 128], BF16, kind="ExternalInput")
    wkT = nc.dram_tensor("wkT", [D, 128], BF16, kind="ExternalInput")
    wvT = nc.dram_tensor("wvT", [D, 128], BF16, kind="ExternalInput")
    woT = nc.dram_tensor("woT", [2, 64, D], BF16, kind="ExternalInput")
    out = nc.dram_tensor("out", [N, D], F32, kind="ExternalOutput")

    with tile.TileContext(nc) as tc, ExitStack() as ctx:
        const = ctx.enter_context(tc.tile_pool(name="const", bufs=1))

        # ---- load inputs (weights first — they gate the first matmul;
        # xT chunked so projections start after chunk 0) ----
        wq_sb = const.tile([128, DCH, 128], BF16)
        nc.sync.dma_start(out=wq_sb, in_=wqT.rearrange("(c p) m -> p c m", p=128))
        wk_sb = const.tile([128, DCH, 128], BF16)
        nc.scalar.dma_start(out=wk_sb, in_=wkT.rearrange("(c p) m -> p c m", p=128))
        wv_sb = const.tile([128, DCH, 128], BF16)
        nc.gpsimd.dma_start(out=wv_sb, in_=wvT.rearrange("(c p) m -> p c m", p=128))
        wo_sb = const.tile([64, 2, D], BF16)
        nc.gpsimd.dma_start(out=wo_sb, in_=woT.rearrange("h d n -> d h n"))
        # xT chunks round-robin the three DMA-capable engine queues so the
        # projection loop never outruns the input transfer
        xT_sb = const.tile([128, DCH, N], BF16)
        engs = (nc.sync, nc.scalar, nc.gpsimd)
        for nchunk in range(8):
            cols = bass.ts(nchunk, N // 8)
            engs[nchunk % 3].dma_start(out=xT_sb[:, :, cols],
                                       in_=xT[:, :, cols].rearrange("c p n -> p c n"))
        ones_sb = const.tile([1, 64], BF16)
        nc.gpsimd.memset(ones_sb, 1.0)

        qT2 = const.tile([128, N], BF16)                   # [2-head d, n]
        kT2 = const.tile([128, N], BF16)
        v2 = const.tile([128, N_KC, 130], BF16)            # [k-part, kc, (v_h0|1|v_h1|1)]
        outT = const.tile([64, 2, N], BF16)                # normalized per-head av

        # ---- projections (copies alternate ACT/DVE to overlap with PE) ----
        with tc.tile_pool(name="proj_psum", bufs=3, space="PSUM") as proj_psum:
            for nt in range(N // 512):
                pq = proj_psum.tile([128, 512], F32, tag="pj")
                for c in range(DCH):
                    nc.tensor.matmul(pq, wq_sb[:, c, :], xT_sb[:, c, bass.ts(nt, 512)],
                                     start=(c == 0), stop=(c == DCH - 1))
                if nt % 2 == 0:
                    nc.scalar.activation(qT2[:, bass.ts(nt, 512)], pq,
                                         mybir.ActivationFunctionType.Copy)
                else:
                    nc.vector.tensor_copy(qT2[:, bass.ts(nt, 512)], pq)
            for nt in range(N // 512):
                pk = proj_psum.tile([128, 512], F32, tag="pj")
                for c in range(DCH):
                    nc.tensor.matmul(pk, wk_sb[:, c, :], xT_sb[:, c, bass.ts(nt, 512)],
                                     start=(c == 0), stop=(c == DCH - 1))
                if nt % 2 == 1:
                    nc.scalar.activation(kT2[:, bass.ts(nt, 512)], pk,
                                         mybir.ActivationFunctionType.Copy)
                else:
                    nc.vector.tensor_copy(kT2[:, bass.ts(nt, 512)], pk)
            # v natural: out[n-tile, hd] = xT-chunk.T @ wv-chunk; 4 kc per psum
            for vt in range(N_KC // 4):
                pv = proj_psum.tile([128, 512], F32, tag="pj")
                for j in range(4):
                    kc = vt * 4 + j
                    for c in range(DCH):
                        nc.tensor.matmul(pv[:, bass.ts(j, 128)],
                                         xT_sb[:, c, bass.ts(kc, 128)], wv_sb[:, c, :],
                                         start=(c == 0), stop=(c == DCH - 1))
                # interleave heads' 64-col halves into v2 (cols 0-63, 65-128)
                src0 = bass.AP(tensor=pv.tensor, offset=pv.offset,
                               ap=[pv.ap[0], [128, 4], [1, 64]])
                dst0 = bass.AP(tensor=v2.tensor, offset=v2.offset + vt * 4 * 130,
                               ap=[v2.ap[0], [130, 4], [1, 64]])
                src1 = bass.AP(tensor=pv.tensor, offset=pv.offset + 64,
                               ap=[pv.ap[0], [128, 4], [1, 64]])
                dst1 = bass.AP(tensor=v2.tensor, offset=v2.offset + vt * 4 * 130 + 65,
                               ap=[v2.ap[0], [130, 4], [1, 64]])
                if vt % 2 == 0:
                    nc.vector.tensor_copy(dst0, src0)
                    nc.scalar.activation(dst1, src1, mybir.ActivationFunctionType.Copy)
                else:
                    nc.scalar.activation(dst0, src0, mybir.ActivationFunctionType.Copy)
                    nc.vector.tensor_copy(dst1, src1)
        # ones columns for the softmax-denominator trick
        nc.gpsimd.memset(v2[:, :, 64], 1.0)
        nc.gpsimd.memset(v2[:, :, 129], 1.0)

        # ---- attention ----
        with (
            tc.tile_pool(name="sc_psum", bufs=5, space="PSUM") as sc_psum,
            tc.tile_pool(name="av_psum", bufs=1, space="PSUM") as av_psum,
            tc.tile_pool(name="op_psum", bufs=1, space="PSUM") as op_psum,
            tc.tile_pool(name="attn_sb", bufs=6) as attn_sb,
            tc.tile_pool(name="norm_sb", bufs=2) as norm_sb,
        ):
            # deferred out-projection and normalize work, interleaved into
            # later kc loops so their PSUM slots / input latencies never
            # block the PE's in-order stream
            op_queue = []
            norm_queue = []

            def issue_norm(base, h, s, rc, avs):
                bh = op_psum.tile([64, 512], F32, tag="po", name=f"bc_{base}_{h}_{s}")
                nc.tensor.matmul(bh, ones_sb, rc[:, bass.ts(s, 512)],
                                 start=True, stop=True)
                nc.vector.tensor_mul(
                    outT[:, h, base + s * 512:base + (s + 1) * 512],
                    avs[:, bass.ts(s, 512)], bh)

            def issue_outproj(base, nt):
                po = op_psum.tile([128, D], F32, tag="po", name=f"po_{base}_{nt}")
                w0 = base + nt * 128
                nc.tensor.matmul(po, outT[:, 0, w0:w0 + 128], wo_sb[:, 0, :],
                                 start=True, stop=False)
                nc.tensor.matmul(po, outT[:, 1, w0:w0 + 128], wo_sb[:, 1, :],
                                 start=False, stop=True)
                ob = norm_sb.tile([128, D], F32, tag="ob", name=f"ob_{base}_{nt}")
                nc.scalar.activation(ob, po, mybir.ActivationFunctionType.Copy)
                nc.sync.dma_start(out=out[w0:w0 + 128, :], in_=ob)

            # last 1024 columns split into two 512-wide windows so the
            # final normalize + out-projection tail is half as deep
            windows = [(0, 1024), (1024, 1024), (2048, 1024),
                       (3072, 512), (3584, 512)]
            for base, width in windows:
                n_s = width // 512
                for h in range(2):
                    av = av_psum.tile([65, width], F32, tag="av",
                                      name=f"av_{base}_{h}")

                    # software-pipelined: sc(k)+exp(k) issue two iterations
                    # ahead of av(k-2) so PE never blocks on an exp in flight
                    def issue_sc_exp(kc):
                        ats = []
                        for s in range(n_s):
                            sc = sc_psum.tile([128, 512], F32, tag="sc",
                                              name=f"sc_{base}_{h}_{kc}_{s}")
                            nc.tensor.matmul(
                                sc,
                                kT2[h * 64:(h + 1) * 64, bass.ts(kc, 128)],
                                qT2[h * 64:(h + 1) * 64,
                                    base + s * 512:base + (s + 1) * 512],
                                start=True, stop=True)
                            at = attn_sb.tile([128, 512], BF16, tag="at",
                                              name=f"at_{base}_{h}_{kc}_{s}")
                            if dve_exp_tile(kc, s, n_s):
                                nc.vector.tensor_scalar(
                                    at.bitcast(I16), sc, EXP_A, EXP_B,
                                    mybir.AluOpType.mult, mybir.AluOpType.add)
                            else:
                                nc.scalar.activation(
                                    at, sc, mybir.ActivationFunctionType.Exp,
                                    scale=float(SCALE))
                            ats.append(at)
                        return ats

                    def issue_av(kc, ats):
                        for s in range(n_s):
                            nc.tensor.matmul(
                                av[:, bass.ts(s, 512)],
                                v2[:, kc, h * 65:(h + 1) * 65],
                                ats[s],
                                start=(kc == 0), stop=(kc == N_KC - 1))

                    at_p2 = issue_sc_exp(0)
                    at_p1 = issue_sc_exp(1)
                    for kc in range(2, N_KC):
                        at_cur = issue_sc_exp(kc)
                        issue_av(kc - 2, at_p2)
                        at_p2, at_p1 = at_p1, at_cur
                        if kc % 8 == 2 and norm_queue:
                            issue_norm(*norm_queue.pop(0))
                        elif kc % 4 == 0 and op_queue:
                            issue_outproj(*op_queue.pop(0))
                    issue_av(N_KC - 2, at_p2)
                    issue_av(N_KC - 1, at_p1)
                    # normalize: outT[h] = av[:64] * 1/av[64].
                    # recip reads PSUM directly; the drain copy splits across
                    # ACT/DVE halves so the av bank frees quickly; the
                    # broadcast matmul + mul are deferred into the next block.
                    rc = norm_sb.tile([1, width], BF16, tag="rc",
                                      name=f"rc_{base}_{h}")
                    with nc.allow_low_precision(
                            reason="bf16 1/denom: ±0.2% uniform per-column "
                                   "scale, well inside the rel-err budget"):
                        nc.vector.reciprocal(rc, av[64:65, :])
                    avs = norm_sb.tile([64, width], F32, tag="avs",
                                       name=f"avs_{base}_{h}")
                    if n_s == 2:
                        nc.scalar.activation(avs[:, 0:512], av[0:64, 0:512],
                                             mybir.ActivationFunctionType.Copy)
                        nc.vector.tensor_copy(avs[:, 512:width],
                                              av[0:64, 512:width])
                    else:
                        nc.scalar.activation(avs, av[0:64, :],
                                             mybir.ActivationFunctionType.Copy)
                    norm_queue.extend((base, h, s, rc, avs) for s in range(n_s))
                op_queue.extend((base, nt) for nt in range(width // 128))
            while norm_queue:
                issue_norm(*norm_queue.pop(0))

        # drain the final q-chunk's output projection with a deeper PSUM
        # ring (the attention pools' banks are free once they close)
        with (
            tc.tile_pool(name="op2_psum", bufs=4, space="PSUM") as op2_psum,
            tc.tile_pool(name="tail_sb", bufs=4) as tail_sb,
        ):
            while op_queue:
                base, nt = op_queue.pop(0)
                po = op2_psum.tile([128, D], F32, tag="po2", name=f"po2_{base}_{nt}")
                w0 = base + nt * 128
                nc.tensor.matmul(po, outT[:, 0, w0:w0 + 128], wo_sb[:, 0, :],
                                 start=True, stop=False)
                nc.tensor.matmul(po, outT[:, 1, w0:w0 + 128], wo_sb[:, 1, :],
                                 start=False, stop=True)
                ob = tail_sb.tile([128, D], F32, tag="ob2", name=f"ob2_{base}_{nt}")
                if nt % 2 == 0:
                    nc.vector.tensor_copy(ob, po)
                else:
                    nc.scalar.activation(ob, po, mybir.ActivationFunctionType.Copy)
                nc.sync.dma_start(out=out[w0:w0 + 128, :], in_=ob)

    nc.compile()
    return nc


_NC_CACHE = None


def build_in_maps(x, Wq, Wk, Wv, Wo):
    bf = ml_dtypes.bfloat16
    x = np.asarray(x, np.float32)
    Wq, Wk, Wv, Wo = (np.asarray(a, np.float32) for a in (Wq, Wk, Wv, Wo))
    in_maps = []
    for c in range(8):
        b = c // 4
        h0 = 2 * (c % 4)
        xTc = np.ascontiguousarray(
            x[b].T.astype(bf).reshape(DCH, 128, N))
        wqT = np.ascontiguousarray(Wq[h0 * 64:(h0 + 2) * 64].T.astype(bf))
        wkT = np.ascontiguousarray(Wk[h0 * 64:(h0 + 2) * 64].T.astype(bf))
        wvT = np.ascontiguousarray(Wv[h0 * 64:(h0 + 2) * 64].T.astype(bf))
        woT = np.stack([np.ascontiguousarray(
            Wo[:, (h0 + h) * 64:(h0 + h + 1) * 64].T.astype(bf)) for h in range(2)])
        in_maps.append({"xT": xTc, "wqT": wqT, "wkT": wkT, "wvT": wvT, "woT": woT})
    return in_maps


def kernel(x, Wq, Wk, Wv, Wo, bo):
    global _NC_CACHE
    bo = np.asarray(bo, np.float32)
    in_maps = build_in_maps(x, Wq, Wk, Wv, Wo)

    if _NC_CACHE is None:
        _NC_CACHE = build_bass()
    res = run_bass_kernel_spmd(_NC_CACHE, in_maps, list(range(8)))
    partials = [np.asarray(res.results[c]["out"], np.float32) for c in range(8)]

    out = np.empty((B, N, D), np.float32)
    for b in range(B):
        out[b] = partials[4 * b] + partials[4 * b + 1] + partials[4 * b + 2] + partials[4 * b + 3] + bo
    return out


if __name__ == "__main__":
    nc = build_bass()
    print("built ok")


# revision 36
# speedup vs baseline: 1.1204x; 1.1204x over previous
"""Self-attention (8 heads, d=64, B=2, N=4096, D=512) on 8 TRN2 NeuronCores.

Sharding: batch*heads across cores — core c handles batch b=c//4, heads
(2*(c%4), 2*(c%4)+1). Projection weights are sliced per-core on the host;
x is pre-transposed on the host so the device needs no transposes at all.

v2 changes vs baseline (403us -> target ~260us):
  - softmax exp split across two engines: ACT does exact exp (bf16 out),
    DVE does a Schraudolph-style exp for kc%5==4 tiles (one fused
    multiply-add writing int16 bf16-bit-patterns; the uniform half-ulp
    bias cancels in the softmax ratio). Measured full-model rel err
    ~7e-3 vs the 2e-2 gate.
  - h-outer attention loop: one live av accumulator (2 PSUM banks)
    frees banks for a dedicated out-projection pool.
  - out-projection runs per-qq inside the loop, DMAs PSUM->DRAM
    directly (no SBUF staging copy).
  - projection-phase PSUM->SBUF copies spread across ACT and DVE.
  - input DMA chunked so projections start after the first chunk.

Device dataflow (per core, fully transposed "scoresT" formulation):
  qT2/kT2 [hd=128, n]  = W.T-chunks @ xT-chunks          (PE)
  v2      [n, hd+ones] natural                            (PE, bf16)
  per head h, per q-chunk qq (1024 wide):
    for kc in 32:  scT psum[128k,1024q] = kh.T @ qh       (PE)
                   attnT = exp(scT*SCALE) -> bf16 SBUF    (ACT or DVE)
                   av[65,1024] += v2'[kc].T @ attnT       (PE, accumulate)
    row 64 of av = softmax denominator (ones column of v2')
    outT[h] = av[:64] * (1/denom)                         (DVE + DMA bcast)
  out[qq] = sum_h outT[h].T @ woT[h]  -> DMA PSUM->DRAM   (PE)
Host: out[b] = sum of its 4 cores' partials + bo.
"""
import math
import numpy as np
import ml_dtypes
from contextlib import ExitStack

import concourse.bass as bass
from concourse import bacc
import concourse.mybir as mybir
import concourse.tile as tile
from concourse.bass_utils import run_bass_kernel_spmd

B, N, D = 2, 4096, 512
HEADS, DH = 8, 64
SCALE = DH ** -0.5

F32 = mybir.dt.float32
F32R = mybir.dt.float32r
BF16 = mybir.dt.bfloat16
I16 = mybir.dt.int16

QQ_W = 1024          # q-chunk width in the attention loop
N_QQ = N // QQ_W     # 4
N_KC = N // 128      # 32 key chunks
DCH = D // 128       # 4 contraction chunks for projections

# Schraudolph exp in bf16 bit space: bits = y*128*log2(e) + (128*127 - s + 0.5)
# with y = score*SCALE folded into the multiplier. s = 128*0.0579 balances
# the max relative error of the piecewise-linear approximation (~±3%).
EXP_A = 128.0 * math.log2(math.e) * SCALE
EXP_B = 128.0 * 127.0 - 128.0 * 0.0579 + 0.5


def dve_exp_tile(kc: int, s: int, n_s: int) -> bool:
    """Which score half-tiles get the approximate DVE exp. Normally the s=1
    half of each kc (minus every 16th, rebalancing DVE's normalize work);
    single-half windows alternate by kc instead."""
    if n_s == 1:
        return kc % 2 == 1
    return s == 1 and kc % 16 != 15


def build_bass():
    nc = bacc.Bacc(None, target_bir_lowering=False)

    xT = nc.dram_tensor("xT", [DCH, 128, N], BF16, kind="ExternalInput")
    wqT = nc.dram_tensor("wqT", [D, 128], BF16, kind="ExternalInput")
    wkT = nc.dram_tensor("wkT", [D, 128], BF16, kind="ExternalInput")
    wvT = nc.dram_tensor("wvT", [D, 128], BF16, kind="ExternalInput")
    woT = nc.dram_tensor("woT", [2, 64, D], BF16, kind="ExternalInput")
    out = nc.dram_tensor("out", [N, D], F32, kind="ExternalOutput")

    with tile.TileContext(nc) as tc, ExitStack() as ctx:
        const = ctx.enter_context(tc.tile_pool(name="const", bufs=1))

        # ---- load inputs (weights first — they gate the first matmul;
        # xT chunked so projections start after chunk 0) ----
        wq_sb = const.tile([128, DCH, 128], BF16)
        nc.sync.dma_start(out=wq_sb, in_=wqT.rearrange("(c p) m -> p c m", p=128))
        wk_sb = const.tile([128, DCH, 128], BF16)
        nc.scalar.dma_start(out=wk_sb, in_=wkT.rearrange("(c p) m -> p c m", p=128))
        # xT chunks round-robin three DMA queues; wv/wo (not needed until
        # ~14us in) queue behind gpsimd's chunks
        xT_sb = const.tile([128, DCH, N], BF16)
        engs = (nc.sync, nc.scalar, nc.gpsimd)
        for nchunk in range(8):
            cols = bass.ts(nchunk, N // 8)
            engs[nchunk % 3].dma_start(out=xT_sb[:, :, cols],
                                       in_=xT[:, :, cols].rearrange("c p n -> p c n"))
        wv_sb = const.tile([128, DCH, 128], BF16)
        nc.gpsimd.dma_start(out=wv_sb, in_=wvT.rearrange("(c p) m -> p c m", p=128))
        wo_sb = const.tile([64, 2, D], BF16)
        nc.gpsimd.dma_start(out=wo_sb, in_=woT.rearrange("h d n -> d h n"))
        ones_sb = const.tile([1, 64], BF16)
        nc.gpsimd.memset(ones_sb, 1.0)

        qT2 = const.tile([128, N], BF16)                   # [2-head d, n]
        kT2 = const.tile([128, N], BF16)
        v2 = const.tile([128, N_KC, 130], BF16)            # [k-part, kc, (v_h0|1|v_h1|1)]
        outT = const.tile([64, 2, N], BF16)                # normalized per-head av

        # ---- projections (copies alternate ACT/DVE to overlap with PE) ----
        with tc.tile_pool(name="proj_psum", bufs=6, space="PSUM") as proj_psum:
            for nt in range(N // 512):
                pq = proj_psum.tile([128, 512], F32, tag="pj")
                for c in range(DCH):
                    nc.tensor.matmul(pq, wq_sb[:, c, :], xT_sb[:, c, bass.ts(nt, 512)],
                                     start=(c == 0), stop=(c == DCH - 1))
                if nt % 2 == 0:
                    nc.scalar.activation(qT2[:, bass.ts(nt, 512)], pq,
                                         mybir.ActivationFunctionType.Copy)
                else:
                    nc.vector.tensor_copy(qT2[:, bass.ts(nt, 512)], pq)
            for nt in range(N // 512):
                pk = proj_psum.tile([128, 512], F32, tag="pj")
                for c in range(DCH):
                    nc.tensor.matmul(pk, wk_sb[:, c, :], xT_sb[:, c, bass.ts(nt, 512)],
                                     start=(c == 0), stop=(c == DCH - 1))
                if nt % 2 == 1:
                    nc.scalar.activation(kT2[:, bass.ts(nt, 512)], pk,
                                         mybir.ActivationFunctionType.Copy)
                else:
                    nc.vector.tensor_copy(kT2[:, bass.ts(nt, 512)], pk)
            # v natural: out[n-tile, hd] = xT-chunk.T @ wv-chunk; 4 kc per psum
            for vt in range(N_KC // 4):
                pv = proj_psum.tile([128, 512], F32, tag="pj")
                for j in range(4):
                    kc = vt * 4 + j
                    for c in range(DCH):
                        nc.tensor.matmul(pv[:, bass.ts(j, 128)],
                                         xT_sb[:, c, bass.ts(kc, 128)], wv_sb[:, c, :],
                                         start=(c == 0), stop=(c == DCH - 1))
                # interleave heads' 64-col halves into v2 (cols 0-63, 65-128)
                src0 = bass.AP(tensor=pv.tensor, offset=pv.offset,
                               ap=[pv.ap[0], [128, 4], [1, 64]])
                dst0 = bass.AP(tensor=v2.tensor, offset=v2.offset + vt * 4 * 130,
                               ap=[v2.ap[0], [130, 4], [1, 64]])
                src1 = bass.AP(tensor=pv.tensor, offset=pv.offset + 64,
                               ap=[pv.ap[0], [128, 4], [1, 64]])
                dst1 = bass.AP(tensor=v2.tensor, offset=v2.offset + vt * 4 * 130 + 65,
                               ap=[v2.ap[0], [130, 4], [1, 64]])
                if vt % 2 == 0:
                    nc.vector.tensor_copy(dst0, src0)
                    nc.scalar.activation(dst1, src1, mybir.ActivationFunctionType.Copy)
                else:
                    nc.scalar.activation(dst0, src0, mybir.ActivationFunctionType.Copy)
                    nc.vector.tensor_copy(dst1, src1)
        # ones columns for the softmax-denominator trick
        nc.gpsimd.memset(v2[:, :, 64], 1.0)
        nc.gpsimd.memset(v2[:, :, 129], 1.0)

        # ---- attention ----
        with (
            tc.tile_pool(name="sc_psum", bufs=5, space="PSUM") as sc_psum,
            tc.tile_pool(name="av_psum", bufs=1, space="PSUM") as av_psum,
            tc.tile_pool(name="op_psum", bufs=1, space="PSUM") as op_psum,
            tc.tile_pool(name="attn_sb", bufs=6) as attn_sb,
            tc.tile_pool(name="norm_sb", bufs=2) as norm_sb,
        ):
            # deferred out-projection and normalize work, interleaved into
            # later kc loops so their PSUM slots / input latencies never
            # block the PE's in-order stream
            op_queue = []
            norm_queue = []

            def issue_norm(base, h, s, rc, avs):
                bh = op_psum.tile([64, 512], F32, tag="po", name=f"bc_{base}_{h}_{s}")
                nc.tensor.matmul(bh, ones_sb, rc[:, bass.ts(s, 512)],
                                 start=True, stop=True)
                nc.vector.tensor_mul(
                    outT[:, h, base + s * 512:base + (s + 1) * 512],
                    avs[:, bass.ts(s, 512)], bh)

            def issue_outproj(base, nt):
                po = op_psum.tile([128, D], F32, tag="po", name=f"po_{base}_{nt}")
                w0 = base + nt * 128
                nc.tensor.matmul(po, outT[:, 0, w0:w0 + 128], wo_sb[:, 0, :],
                                 start=True, stop=False)
                nc.tensor.matmul(po, outT[:, 1, w0:w0 + 128], wo_sb[:, 1, :],
                                 start=False, stop=True)
                ob = norm_sb.tile([128, D], F32, tag="ob", name=f"ob_{base}_{nt}")
                nc.scalar.activation(ob, po, mybir.ActivationFunctionType.Copy)
                nc.sync.dma_start(out=out[w0:w0 + 128, :], in_=ob)

            # last 1024 columns split into two 512-wide windows so the
            # final normalize + out-projection tail is half as deep
            windows = [(0, 1024), (1024, 1024), (2048, 1024),
                       (3072, 512), (3584, 512)]
            for base, width in windows:
                n_s = width // 512
                for h in range(2):
                    av = av_psum.tile([65, width], F32, tag="av",
                                      name=f"av_{base}_{h}")

                    # software-pipelined: sc(k)+exp(k) issue two iterations
                    # ahead of av(k-2) so PE never blocks on an exp in flight
                    def issue_sc_exp(kc):
                        ats = []
                        for s in range(n_s):
                            sc = sc_psum.tile([128, 512], F32, tag="sc",
                                              name=f"sc_{base}_{h}_{kc}_{s}")
                            nc.tensor.matmul(
                                sc,
                                kT2[h * 64:(h + 1) * 64, bass.ts(kc, 128)],
                                qT2[h * 64:(h + 1) * 64,
                                    base + s * 512:base + (s + 1) * 512],
                                start=True, stop=True)
                            at = attn_sb.tile([128, 512], BF16, tag="at",
                                              name=f"at_{base}_{h}_{kc}_{s}")
                            if dve_exp_tile(kc, s, n_s):
                                nc.vector.tensor_scalar(
                                    at.bitcast(I16), sc, EXP_A, EXP_B,
                                    mybir.AluOpType.mult, mybir.AluOpType.add)
                            else:
                                nc.scalar.activation(
                                    at, sc, mybir.ActivationFunctionType.Exp,
                                    scale=float(SCALE))
                            ats.append(at)
                        return ats

                    def issue_av(kc, ats):
                        for s in range(n_s):
                            nc.tensor.matmul(
                                av[:, bass.ts(s, 512)],
                                v2[:, kc, h * 65:(h + 1) * 65],
                                ats[s],
                                start=(kc == 0), stop=(kc == N_KC - 1))

                    at_p2 = issue_sc_exp(0)
                    at_p1 = issue_sc_exp(1)
                    for kc in range(2, N_KC):
                        at_cur = issue_sc_exp(kc)
                        issue_av(kc - 2, at_p2)
                        at_p2, at_p1 = at_p1, at_cur
                        if kc % 8 == 2 and norm_queue:
                            issue_norm(*norm_queue.pop(0))
                        elif kc % 4 == 0 and op_queue:
                            issue_outproj(*op_queue.pop(0))
                    issue_av(N_KC - 2, at_p2)
                    issue_av(N_KC - 1, at_p1)
                    # normalize: outT[h] = av[:64] * 1/av[64].
                    # recip reads PSUM directly; the drain copy splits across
                    # ACT/DVE halves so the av bank frees quickly; the
                    # broadcast matmul + mul are deferred into the next block.
                    rc = norm_sb.tile([1, width], BF16, tag="rc",
                                      name=f"rc_{base}_{h}")
                    with nc.allow_low_precision(
                            reason="bf16 1/denom: ±0.2% uniform per-column "
                                   "scale, well inside the rel-err budget"):
                        nc.vector.reciprocal(rc, av[64:65, :])
                    avs = norm_sb.tile([64, width], F32, tag="avs",
                                       name=f"avs_{base}_{h}")
                    if n_s == 2:
                        nc.scalar.activation(avs[:, 0:512], av[0:64, 0:512],
                                             mybir.ActivationFunctionType.Copy)
                        nc.vector.tensor_copy(avs[:, 512:width],
                                              av[0:64, 512:width])
                    else:
                        nc.scalar.activation(avs, av[0:64, :],
                                             mybir.ActivationFunctionType.Copy)
                    norm_queue.extend((base, h, s, rc, avs) for s in range(n_s))
                op_queue.extend((base, nt) for nt in range(width // 128))
            while norm_queue:
                issue_norm(*norm_queue.pop(0))

        # drain the final q-chunk's output projection with a deeper PSUM
        # ring (the attention pools' banks are free once they close)
        with (
            tc.tile_pool(name="op2_psum", bufs=4, space="PSUM") as op2_psum,
            tc.tile_pool(name="tail_sb", bufs=4) as tail_sb,
        ):
            while op_queue:
                base, nt = op_queue.pop(0)
                po = op2_psum.tile([128, D], F32, tag="po2", name=f"po2_{base}_{nt}")
                w0 = base + nt * 128
                nc.tensor.matmul(po, outT[:, 0, w0:w0 + 128], wo_sb[:, 0, :],
                                 start=True, stop=False)
                nc.tensor.matmul(po, outT[:, 1, w0:w0 + 128], wo_sb[:, 1, :],
                                 start=False, stop=True)
                ob = tail_sb.tile([128, D], F32, tag="ob2", name=f"ob2_{base}_{nt}")
                if nt % 2 == 0:
                    nc.vector.tensor_copy(ob, po)
                else:
                    nc.scalar.activation(ob, po, mybir.ActivationFunctionType.Copy)
                nc.sync.dma_start(out=out[w0:w0 + 128, :], in_=ob)

    nc.compile()
    return nc


_NC_CACHE = None


def build_in_maps(x, Wq, Wk, Wv, Wo):
    bf = ml_dtypes.bfloat16
    x = np.asarray(x, np.float32)
    Wq, Wk, Wv, Wo = (np.asarray(a, np.float32) for a in (Wq, Wk, Wv, Wo))
    in_maps = []
    for c in range(8):
        b = c // 4
        h0 = 2 * (c % 4)
        xTc = np.ascontiguousarray(
            x[b].T.astype(bf).reshape(DCH, 128, N))
        wqT = np.ascontiguousarray(Wq[h0 * 64:(h0 + 2) * 64].T.astype(bf))
        wkT = np.ascontiguousarray(Wk[h0 * 64:(h0 + 2) * 64].T.astype(bf))
        wvT = np.ascontiguousarray(Wv[h0 * 64:(h0 + 2) * 64].T.astype(bf))
        woT = np.stack([np.ascontiguousarray(
            Wo[:, (h0 + h) * 64:(h0 + h + 1) * 64].T.astype(bf)) for h in range(2)])
        in_maps.append({"xT": xTc, "wqT": wqT, "wkT": wkT, "wvT": wvT, "woT": woT})
    return in_maps


def kernel(x, Wq, Wk, Wv, Wo, bo):
    global _NC_CACHE
    bo = np.asarray(bo, np.float32)
    in_maps = build_in_maps(x, Wq, Wk, Wv, Wo)

    if _NC_CACHE is None:
        _NC_CACHE = build_bass()
    res = run_bass_kernel_spmd(_NC_CACHE, in_maps, list(range(8)))
    partials = [np.asarray(res.results[c]["out"], np.float32) for c in range(8)]

    out = np.empty((B, N, D), np.float32)
    for b in range(B):
        out[b] = partials[4 * b] + partials[4 * b + 1] + partials[4 * b + 2] + partials[4 * b + 3] + bo
    return out


if __name__ == "__main__":
    nc = build_bass()
    print("built ok")
